# revision 1
# baseline (speedup 1.0000x reference)
"""Graphormer attention head on 8 trn2 NeuronCores (row-parallel).

out = softmax(mask(q@k.T/8, adj)) @ v  with q/k/v = x@W+b, adj scattered
from edge_index.

Sharding: core c owns output rows [c*1024, (c+1)*1024). k/v are computed
replicated on every core from a streamed x^T. The adjacency mask is
precomputed on the host as {0,1} fp8 (transposed, per-core column slice)
and applied multiplicatively AFTER exp: exp(S)*A equals the masked-softmax
numerator exactly (scores are bounded, ~|S|<8, so no row-max subtraction
is needed and exp never overflows; non-edges contribute exactly 0).
The softmax denominator comes free via a ones-column appended to V.
"""
import os
import sys

for _p in ("/opt/trn_rl_repo", "/root/.axon_site/_ro/trn_rl_repo"):
    if os.path.isdir(_p) and _p not in sys.path:
        sys.path.insert(0, _p)

import numpy as np
import ml_dtypes

import concourse.bass as bass
import concourse.bacc as bacc
import concourse.mybir as mybir
import concourse.tile as tile
from concourse.bass_utils import run_bass_kernel_spmd

N = 8192
DIN = 256
DQ = 64
NCORES = 8
NLOC = N // NCORES          # 1024 rows per core
JT = N // 128               # 64 column tiles of 128
SEG = 512                   # fp32 moving-operand max
F32 = mybir.dt.float32
FP8 = mybir.dt.float8e4
AO = None                   # AluOpType, set on import below


def _emit(nc, tc, ctx):
    import contextlib
    from concourse.mybir import AluOpType as AO, ActivationFunctionType as AF

    xt = nc.dram_tensor("xt", [DIN, N], F32, kind="ExternalInput")
    xtq = nc.dram_tensor("xtq", [DIN, NLOC], F32, kind="ExternalInput")
    wq = nc.dram_tensor("wq", [DIN, DQ], F32, kind="ExternalInput")
    wk = nc.dram_tensor("wk", [DIN, DQ], F32, kind="ExternalInput")
    wv = nc.dram_tensor("wv", [DIN, DQ], F32, kind="ExternalInput")
    bq = nc.dram_tensor("bq", [DQ, 1], F32, kind="ExternalInput")
    bk = nc.dram_tensor("bk", [DQ, 1], F32, kind="ExternalInput")
    i65 = nc.dram_tensor("i65", [DQ + 1, DQ + 1], F32, kind="ExternalInput")
    maskt = nc.dram_tensor("maskt", [N, NLOC], FP8, kind="ExternalInput")
    out = nc.dram_tensor("out", [NLOC, DQ], F32, kind="ExternalOutput")

    pers = ctx.enter_context(tc.tile_pool(name="pers", bufs=1))
    pm = ctx.enter_context(tc.tile_pool(name="pm", bufs=4))
    pe_ = ctx.enter_context(tc.tile_pool(name="pe", bufs=3))
    pw = ctx.enter_context(tc.tile_pool(name="pw", bufs=3))
    pfin = ctx.enter_context(tc.tile_pool(name="pfin", bufs=2))
    ps = ctx.enter_context(tc.tile_pool(name="ps", bufs=2, space="PSUM"))
    pacc = ctx.enter_context(tc.tile_pool(name="pacc", bufs=1, space="PSUM"))
    pp = ctx.enter_context(tc.tile_pool(name="pp", bufs=2, space="PSUM"))

    # ---- persistent SBUF ----
    xt_sb = [pers.tile([128, N], F32, tag=f"xt{c}", name=f"xt{c}") for c in range(2)]
    xtq_sb = [pers.tile([128, NLOC], F32, tag=f"xtq{c}", name=f"xtq{c}") for c in range(2)]
    w_sb = {}
    for nm, t in (("wq", wq), ("wk", wk), ("wv", wv)):
        for c in range(2):
            w_sb[nm, c] = pers.tile([128, DQ], F32, tag=f"{nm}{c}", name=f"w{nm}{c}")
            nc.sync.dma_start(w_sb[nm, c][:], t[c * 128:(c + 1) * 128, :])
    bq_sb = pers.tile([DQ, 1], F32, tag="bq")
    bk_sb = pers.tile([DQ, 1], F32, tag="bk")
    i65_sb = pers.tile([DQ + 1, DQ + 1], F32, tag="i65")
    nc.sync.dma_start(bq_sb[:], bq[:])
    nc.sync.dma_start(bk_sb[:], bk[:])
    nc.sync.dma_start(i65_sb[:], i65[:])
    F16 = mybir.dt.float16
    qth_sb = pers.tile([DQ, NLOC], F16, tag="qth")
    qtl_sb = pers.tile([DQ, NLOC], F16, tag="qtl")
    kth_sb = pers.tile([DQ, N], F16, tag="kth")
    ktl_sb = pers.tile([DQ, N], F16, tag="ktl")
    vh_sb = pers.tile([128, JT * (DQ + 1)], F16, tag="vh")
    accT_sb = pers.tile([DQ + 1, NLOC], F32, tag="accT")

    # x^T streamed in 512-col segments so projections can start early
    for c in range(2):
        for s in range(N // SEG):
            nc.sync.dma_start(
                xt_sb[c][:, s * SEG:(s + 1) * SEG],
                xt[c * 128:(c + 1) * 128, s * SEG:(s + 1) * SEG],
            )
        nc.sync.dma_start(xtq_sb[c][:], xtq[c * 128:(c + 1) * 128, :])

    # ---- projections ----
    # Q^T / K^T in fp16 hi+lo pairs (hi = round(q), lo = round(q - hi)) so
    # the scores matmul can run as a 3-term fp16 split (error ~2^-22).
    def _proj_hilo(w_name, xs, ncols, hi, lo, bias):
        for s in range(ncols // SEG):
            t = pp.tile([128, SEG], F32, tag="pp", name=f"pp_{w_name}{s}")
            tp = t[:DQ, :]
            nc.tensor.matmul(tp, w_sb[w_name, 0][:], xs[0][:, s * SEG:(s + 1) * SEG],
                             start=True, stop=False)
            nc.tensor.matmul(tp, w_sb[w_name, 1][:], xs[1][:, s * SEG:(s + 1) * SEG],
                             start=False, stop=True)
            dst = slice(s * SEG, (s + 1) * SEG)
            nc.vector.tensor_scalar_add(hi[:, dst], tp, bias)
            nc.vector.scalar_tensor_tensor(lo[:, dst], tp, bias, hi[:, dst],
                                           AO.add, AO.subtract)

    _proj_hilo("wq", xtq_sb, NLOC, qth_sb, qtl_sb, bq_sb[:])
    _proj_hilo("wk", xt_sb, N, kth_sb, ktl_sb, bk_sb[:])
    # V [8192 x 64] stored j-major as 64 blocks of [128 x 65] (65th col = 1.0
    # for the softmax denominator). bv is folded in at the end via i65.
    vh3 = vh_sb[:].rearrange("p (b e) -> p b e", e=DQ + 1)
    nc.vector.memset(vh3[:, :, DQ:DQ + 1], 1.0)
    for g in range(8):
        t = pp.tile([128, SEG], F32, tag="pp")
        for b in range(8):
            jt = g * 8 + b
            o = t[:, b * DQ:(b + 1) * DQ]
            nc.tensor.matmul(o, xt_sb[0][:, jt * 128:(jt + 1) * 128],
                             w_sb["wv", 0][:], start=True, stop=False)
            nc.tensor.matmul(o, xt_sb[1][:, jt * 128:(jt + 1) * 128],
                             w_sb["wv", 1][:], start=False, stop=True)
        gh = vh3[:, g * 8:(g + 1) * 8, 0:DQ]
        nc.scalar.activation(gh, t[:], AF.Copy)

    # ---- main loop over 64 column tiles ----
    acc = pacc.tile([DQ + 1, NLOC], F32, tag="acc")
    for jt in range(JT):
        m_t = pm.tile([128, NLOC], FP8, tag="m")
        nc.sync.dma_start(m_t[:], maskt[jt * 128:(jt + 1) * 128, :])
        s_t = ps.tile([128, NLOC], F32, tag="s")
        kh = kth_sb[:, jt * 128:(jt + 1) * 128]
        kl = ktl_sb[:, jt * 128:(jt + 1) * 128]
        # 3-term fp16 split; kh stays loaded for 4 matmuls
        for h in range(2):
            hs = slice(h * SEG, (h + 1) * SEG)
            nc.tensor.matmul(s_t[:, hs], kh, qth_sb[:, hs],
                             start=True, stop=False)
            nc.tensor.matmul(s_t[:, hs], kh, qtl_sb[:, hs],
                             start=False, stop=False)
        for h in range(2):
            hs = slice(h * SEG, (h + 1) * SEG)
            nc.tensor.matmul(s_t[:, hs], kl, qth_sb[:, hs],
                             start=False, stop=True)
        e_t = pe_.tile([128, NLOC], F16, tag="e")
        nc.scalar.activation(e_t[:], s_t[:], AF.Exp)
        w_t = pw.tile([128, NLOC], F16, tag="w")
        nc.vector.scalar_tensor_tensor(w_t[:], e_t[:], 1.0, m_t[:],
                                       AO.mult, AO.mult)
        vhb = vh3[:, jt, :]
        for h in range(2):
            hs = slice(h * SEG, (h + 1) * SEG)
            nc.tensor.matmul(acc[:, hs], vhb, w_t[:, hs],
                             start=(jt == 0), stop=(jt == JT - 1))

    # ---- finish: transpose via matmul with I65 (adds bv*Z), divide by Z ----
    nc.scalar.activation(accT_sb[:], acc[:], AF.Copy)
    for it in range(NLOC // 128):
        po = pp.tile([128, DQ + 1], F32, tag="pp")
        nc.tensor.matmul(po[:], accT_sb[:, it * 128:(it + 1) * 128], i65_sb[:],
                         start=True, stop=True)
        rz = pfin.tile([128, 1], F32, tag="rz")
        nc.vector.reciprocal(rz[:], po[:, DQ:DQ + 1])
        o_t = pfin.tile([128, DQ], F32, tag="o")
        nc.vector.tensor_scalar_mul(o_t[:], po[:, 0:DQ], rz[:])
        nc.sync.dma_start(out[it * 128:(it + 1) * 128, :], o_t[:])


_CACHE = {}


def _program():
    if "nc" not in _CACHE:
        import contextlib
        nc = bacc.Bacc("TRN2", target_bir_lowering=False, debug=False,
                       num_devices=NCORES)
        with tile.TileContext(nc) as tc:
            with contextlib.ExitStack() as ctx:
                _emit(nc, tc, ctx)
        nc.compile()
        _CACHE["nc"] = nc
    return _CACHE["nc"]


def kernel(**inputs):
    x = np.asarray(inputs["x"], dtype=np.float32)
    ei = np.asarray(inputs["edge_index"])
    Wq = np.asarray(inputs["Wq"], dtype=np.float32)
    bq = np.asarray(inputs["bq"], dtype=np.float32)
    Wk = np.asarray(inputs["Wk"], dtype=np.float32)
    bk = np.asarray(inputs["bk"], dtype=np.float32)
    Wv = np.asarray(inputs["Wv"], dtype=np.float32)
    bv = np.asarray(inputs["bv"], dtype=np.float32)

    scale = 1.0 / np.sqrt(np.float32(DQ))
    xT = np.ascontiguousarray(x.T)                      # (256, 8192)
    wq_s = np.ascontiguousarray(Wq * scale)
    bq_s = np.ascontiguousarray((bq * scale).reshape(DQ, 1))
    bk_c = np.ascontiguousarray(bk.reshape(DQ, 1))
    i65 = np.eye(DQ + 1, dtype=np.float32)
    i65[DQ, :DQ] = bv
    adj = np.zeros((N, N), dtype=np.bool_)
    adj[ei[0], ei[1]] = True

    in_maps = []
    for c in range(NCORES):
        rows = slice(c * NLOC, (c + 1) * NLOC)
        in_maps.append({
            "xt": xT,
            "xtq": np.ascontiguousarray(xT[:, rows]),
            "wq": wq_s, "wk": Wk, "wv": Wv,
            "bq": bq_s, "bk": bk_c, "i65": i65,
            "maskt": np.ascontiguousarray(adj[rows].T).astype(
                ml_dtypes.float8_e4m3),
        })

    global _last_in_maps
    _last_in_maps = in_maps
    nc = _program()
    res = run_bass_kernel_spmd(nc, in_maps, core_ids=list(range(NCORES)))
    out = np.concatenate([res.results[c]["out"] for c in range(NCORES)], axis=0)
    return out.astype(np.float32)


_last_in_maps = None

